# revision 1
# baseline (speedup 1.0000x reference)
"""GQA attention (RoPE + softmax + o_proj) on 8 Trainium2 NeuronCores.

Problem shapes (hardcoded): hidden_states [4, 2048, 2048], 16 q heads,
4 kv heads, head_dim 128, rope theta inputs given as cos/sin tables.

Sharding: core c -> (batch b = c // 2, q-head half = c % 2).  Each core
computes 8 q heads + their 2 kv heads for one batch and produces a
partial o_proj output [2048, 2048]; the host sums the two halves per
batch (tensor parallel, no device collectives).

All matmuls run in fp16 (1 cycle/row on PE) with fp32 PSUM accumulation:
  - q^T/k^T = W^T @ hs^T with hid on partitions (weights are natural lhsT)
  - RoPE via a +-1 permutation matmul (rotate_half) + DVE combine
  - scores^T[t, s] with k^T tiles stationary; exp via ScalarE (fused
    1/sqrt(d) scale) writes P^T fp16 straight to SBUF
  - softmax denominators via an all-ones stationary matmul over P^T
    (result is replicated across partitions = pre-broadcast), DVE
    reciprocal, and one fused normalize+cast on PSUM->SBUF copyback
  - attn^T[d, s] = v-tiles stationary @ P^T; o_proj with attn^T tiles
    stationary over Wo.
"""

import sys

import numpy as np

B, S, HID = 4, 2048, 2048
NH, NKV, HD = 16, 4, 128
NH_L = 8        # q heads per core
NKV_L = 2       # kv heads per core
GROUP = NH // NKV
P = 128
ST = 512        # s-block (matmul free dim)
NSB = S // ST   # 4 s-blocks
KT = HID // P   # 16 contraction tiles over hidden
TT = S // P     # 16 key/t tiles
SCALE = 1.0 / float(np.sqrt(HD))

_CACHE = {}


def _rot_lhsT():
    """Stationary operand R such that R.T @ q^T = rotate_half(q)^T."""
    r = np.zeros((HD, HD), dtype=np.float16)
    half = HD // 2
    i = np.arange(half)
    r[i + half, i] = -1.0
    r[i, i + half] = 1.0
    return r


def _build():
    if "/opt/trn_rl_repo" not in sys.path:
        sys.path.insert(0, "/opt/trn_rl_repo")
    import concourse.mybir as mybir
    from concourse import bacc
    from concourse.tile import TileContext

    dt = mybir.dt
    f16, f32 = dt.float16, dt.float32

    nc = bacc.Bacc("TRN2", target_bir_lowering=False, debug=False, num_devices=8)
    hsT = nc.dram_tensor("hsT", [HID, S], f16, kind="ExternalInput").ap()
    wq = nc.dram_tensor("wq", [HID, NH_L * HD], f16, kind="ExternalInput").ap()
    wk = nc.dram_tensor("wk", [HID, NKV_L * HD], f16, kind="ExternalInput").ap()
    wv = nc.dram_tensor("wv", [HID, NKV_L * HD], f16, kind="ExternalInput").ap()
    wo = nc.dram_tensor("wo", [NH_L * HD, HID], f16, kind="ExternalInput").ap()
    cosT = nc.dram_tensor("cosT", [HD, S], f16, kind="ExternalInput").ap()
    sinT = nc.dram_tensor("sinT", [HD, S], f16, kind="ExternalInput").ap()
    rmat = nc.dram_tensor("rmat", [HD, HD], f16, kind="ExternalInput").ap()
    out = nc.dram_tensor("out", [S, HID], f32, kind="ExternalOutput").ap()

    EXP = mybir.ActivationFunctionType.Exp

    with TileContext(nc) as tc:
        with (
            tc.tile_pool(name="consts", bufs=1) as consts,
            tc.tile_pool(name="qkv", bufs=1) as qkvp,
            tc.tile_pool(name="smalls", bufs=4) as smalls,
            tc.tile_pool(name="ps", bufs=8, space="PSUM") as ps,
        ):
            ones = consts.tile([P, P], f16, tag="ones")
            nc.vector.memset(ones, 1.0)
            rot = consts.tile([HD, HD], f16, tag="rot")
            nc.sync.dma_start(out=rot, in_=rmat)
            cos_sb = consts.tile([HD, S], f16, tag="cos")
            nc.sync.dma_start(out=cos_sb, in_=cosT)
            sin_sb = consts.tile([HD, S], f16, tag="sin")
            nc.sync.dma_start(out=sin_sb, in_=sinT)

            q_sb = qkvp.tile([P, NH_L, S], f16, tag="q")
            k_sb = qkvp.tile([P, NKV_L, S], f16, tag="k")
            v_sb = qkvp.tile([P, TT, NKV_L * HD], f16, tag="v")

            # ---------------- Phase A: projections + RoPE ----------------
            with (
                tc.tile_pool(name="wqkv", bufs=1) as wp,
                tc.tile_pool(name="hs", bufs=2) as hsp,
            ):
                wq_sb = wp.tile([P, KT, NH_L * HD], f16, tag="wq")
                nc.sync.dma_start(out=wq_sb, in_=wq.rearrange("(kt p) f -> p kt f", p=P))
                wk_sb = wp.tile([P, KT, NKV_L * HD], f16, tag="wk")
                nc.sync.dma_start(out=wk_sb, in_=wk.rearrange("(kt p) f -> p kt f", p=P))
                wv_sb = wp.tile([P, KT, NKV_L * HD], f16, tag="wv")
                nc.sync.dma_start(out=wv_sb, in_=wv.rearrange("(kt p) f -> p kt f", p=P))

                for si in range(NSB):
                    s0 = si * ST
                    hs_blk = hsp.tile([P, KT, ST], f16, tag="hs")
                    nc.sync.dma_start(
                        out=hs_blk,
                        in_=hsT[:, s0 : s0 + ST].rearrange("(kt p) s -> p kt s", p=P),
                    )

                    def proj_rope(w_sb, ft, dst, dsti):
                        pm = ps.tile([P, ST], f32, tag="ps")
                        for kt in range(KT):
                            nc.tensor.matmul(
                                pm,
                                lhsT=w_sb[:, kt, ft * HD : (ft + 1) * HD],
                                rhs=hs_blk[:, kt, :],
                                start=(kt == 0),
                                stop=(kt == KT - 1),
                            )
                        qc = smalls.tile([P, ST], f16, tag="qc")
                        nc.scalar.copy(qc, pm)
                        pr = ps.tile([P, ST], f32, tag="ps")
                        nc.tensor.matmul(pr, lhsT=rot, rhs=qc, start=True, stop=True)
                        rc = smalls.tile([P, ST], f16, tag="rc")
                        nc.scalar.copy(rc, pr)
                        t1 = smalls.tile([P, ST], f16, tag="t1")
                        nc.vector.tensor_mul(t1, qc, cos_sb[:, s0 : s0 + ST])
                        t2 = smalls.tile([P, ST], f16, tag="t2")
                        nc.vector.tensor_mul(t2, rc, sin_sb[:, s0 : s0 + ST])
                        nc.vector.tensor_add(dst[:, dsti, s0 : s0 + ST], t1, t2)

                    for h in range(NH_L):
                        proj_rope(wq_sb, h, q_sb, h)
                    for j in range(NKV_L):
                        proj_rope(wk_sb, j, k_sb, j)

                    # v in natural [t, d] layout: hs tiles stationary
                    for sj in range(ST // P):
                        tt = s0 // P + sj
                        pv = ps.tile([P, NKV_L * HD], f32, tag="ps")
                        for kt in range(KT):
                            nc.tensor.matmul(
                                pv,
                                lhsT=hs_blk[:, kt, sj * P : (sj + 1) * P],
                                rhs=wv_sb[:, kt, :],
                                start=(kt == 0),
                                stop=(kt == KT - 1),
                            )
                        nc.scalar.copy(v_sb[:, tt, :], pv)

            # ---------------- Phase B: attention ----------------
            with (
                tc.tile_pool(name="wo", bufs=1) as wop,
                tc.tile_pool(name="attn", bufs=1) as ap_,
                tc.tile_pool(name="pblk", bufs=2) as pp,
                tc.tile_pool(name="outp", bufs=2) as op_,
            ):
                wo_sb = wop.tile([P, NH_L, HID], f16, tag="wo")
                nc.sync.dma_start(out=wo_sb, in_=wo.rearrange("(ft p) h -> p ft h", p=P))
                attnT = ap_.tile([P, NH_L, S], f16, tag="attnT")

                for h in range(NH_L):
                    j = h // GROUP
                    for si in range(NSB):
                        s0 = si * ST
                        pblk = pp.tile([P, TT, ST], f16, tag="pblk")
                        for tt in range(TT):
                            psc = ps.tile([P, ST], f32, tag="ps")
                            nc.tensor.matmul(
                                psc,
                                lhsT=k_sb[:, j, tt * P : (tt + 1) * P],
                                rhs=q_sb[:, h, s0 : s0 + ST],
                                start=True,
                                stop=True,
                            )
                            nc.scalar.activation(
                                out=pblk[:, tt, :], in_=psc, func=EXP, scale=SCALE
                            )
                        pcs = ps.tile([P, ST], f32, tag="ps")
                        for tt in range(TT):
                            nc.tensor.matmul(
                                pcs,
                                lhsT=ones,
                                rhs=pblk[:, tt, :],
                                start=(tt == 0),
                                stop=(tt == TT - 1),
                            )
                        rcp = smalls.tile([P, ST], f32, tag="rcp")
                        nc.vector.reciprocal(rcp, pcs)
                        pat = ps.tile([P, ST], f32, tag="ps")
                        for tt in range(TT):
                            nc.tensor.matmul(
                                pat,
                                lhsT=v_sb[:, tt, j * HD : (j + 1) * HD],
                                rhs=pblk[:, tt, :],
                                start=(tt == 0),
                                stop=(tt == TT - 1),
                            )
                        nc.vector.tensor_mul(attnT[:, h, s0 : s0 + ST], pat, rcp)

                # ---------------- Phase C: o_proj ----------------
                for st in range(S // P):
                    ob = op_.tile([P, HID], f32, tag="ob")
                    for ni in range(HID // ST):
                        po = ps.tile([P, ST], f32, tag="ps")
                        for ft in range(NH_L):
                            nc.tensor.matmul(
                                po,
                                lhsT=attnT[:, ft, st * P : (st + 1) * P],
                                rhs=wo_sb[:, ft, ni * ST : (ni + 1) * ST],
                                start=(ft == 0),
                                stop=(ft == NH_L - 1),
                            )
                        nc.vector.tensor_copy(ob[:, ni * ST : (ni + 1) * ST], po)
                    nc.sync.dma_start(out=out[st * P : (st + 1) * P, :], in_=ob)

    nc.compile()
    return nc


def _get_nc():
    if "nc" not in _CACHE:
        _CACHE["nc"] = _build()
    return _CACHE["nc"]


def kernel(hidden_states, cos, sin, Wq, Wk, Wv, Wo):
    if "/opt/trn_rl_repo" not in sys.path:
        sys.path.insert(0, "/opt/trn_rl_repo")
    from concourse.bass_utils import run_bass_kernel_spmd

    hidden_states = np.asarray(hidden_states, dtype=np.float32)
    cos = np.asarray(cos, dtype=np.float32)
    sin = np.asarray(sin, dtype=np.float32)
    Wq = np.asarray(Wq, dtype=np.float32)
    Wk = np.asarray(Wk, dtype=np.float32)
    Wv = np.asarray(Wv, dtype=np.float32)
    Wo = np.asarray(Wo, dtype=np.float32)

    nc = _get_nc()
    rm = _rot_lhsT()

    in_maps = []
    hsT_b = [np.ascontiguousarray(hidden_states[b].T).astype(np.float16) for b in range(B)]
    cosT_b = [np.ascontiguousarray(cos[b].T).astype(np.float16) for b in range(B)]
    sinT_b = [np.ascontiguousarray(sin[b].T).astype(np.float16) for b in range(B)]
    for c in range(2 * B):
        b, half = c // 2, c % 2
        fq = slice(half * NH_L * HD, (half + 1) * NH_L * HD)
        fkv = slice(half * NKV_L * HD, (half + 1) * NKV_L * HD)
        in_maps.append(
            {
                "hsT": hsT_b[b],
                "wq": np.ascontiguousarray(Wq[:, fq]).astype(np.float16),
                "wk": np.ascontiguousarray(Wk[:, fkv]).astype(np.float16),
                "wv": np.ascontiguousarray(Wv[:, fkv]).astype(np.float16),
                "wo": np.ascontiguousarray(Wo[fq, :]).astype(np.float16),
                "cosT": cosT_b[b],
                "sinT": sinT_b[b],
                "rmat": rm,
            }
        )

    res = run_bass_kernel_spmd(nc, in_maps, list(range(2 * B)))
    _CACHE["last_results"] = res

    out = np.empty((B, S, HID), dtype=np.float32)
    for b in range(B):
        out[b] = res.results[2 * b]["out"] + res.results[2 * b + 1]["out"]
    return out


# revision 2
# speedup vs baseline: 1.0592x; 1.0592x over previous
"""GQA attention (RoPE + softmax + o_proj) on 8 Trainium2 NeuronCores.

Problem shapes (hardcoded): hidden_states [4, 2048, 2048], 16 q heads,
4 kv heads, head_dim 128, rope cos/sin tables given as inputs.

Sharding: core c -> (batch b = c // 2, q-head half = c % 2).  Each core
computes 8 q heads + their 2 kv heads for one batch and produces a
partial o_proj output [2048, 2048]; the host sums the two halves per
batch (tensor parallel, no device collectives).

All matmuls run in fp16 (1 cycle/row on PE) with fp32 PSUM accumulation:
  - q^T/k^T = W^T @ hs^T with hid on partitions (weights are natural lhsT)
  - RoPE via a +-1 permutation matmul (rotate_half), software-pipelined
    one head behind the projections so PE never waits on ScalarE
  - scores^T[t, s] with k^T tiles stationary; exp via ScalarE (fused
    1/sqrt(d) scale) writes P^T fp16 straight to SBUF, two PSUM banks
    per activation instruction
  - softmax denominators: one fp16 DVE pairwise add over P^T halves,
    then an all-ones stationary matmul (result replicated across
    partitions = pre-broadcast), DVE reciprocal, fused normalize+cast
    on the attn PSUM->SBUF copyback
  - attn^T[d, s] = v-tiles stationary @ P^T; o_proj with attn^T tiles
    stationary over Wo.
"""

import sys

import numpy as np

B, S, HID = 4, 2048, 2048
NH, NKV, HD = 16, 4, 128
NH_L = 8        # q heads per core
NKV_L = 2       # kv heads per core
GROUP = NH // NKV
P = 128
ST = 512        # s-block (matmul free dim)
NSB = S // ST   # 4 s-blocks
KT = HID // P   # 16 contraction tiles over hidden
TT = S // P     # 16 key/t tiles
SCALE = 1.0 / float(np.sqrt(HD))

_CACHE = {}


def _rot_lhsT():
    """Stationary operand R such that R.T @ q^T = rotate_half(q)^T."""
    r = np.zeros((HD, HD), dtype=np.float16)
    half = HD // 2
    i = np.arange(half)
    r[i + half, i] = -1.0
    r[i, i + half] = 1.0
    return r


def _build():
    if "/opt/trn_rl_repo" not in sys.path:
        sys.path.insert(0, "/opt/trn_rl_repo")
    import concourse.mybir as mybir
    from concourse import bacc
    from concourse.tile import TileContext

    dt = mybir.dt
    f16, f32 = dt.float16, dt.float32

    nc = bacc.Bacc("TRN2", target_bir_lowering=False, debug=False, num_devices=8)
    # host-pretiled layouts (see kernel() below)
    hsT = nc.dram_tensor("hsT", [P, KT, S], f16, kind="ExternalInput").ap()
    wq = nc.dram_tensor("wq", [P, NH_L, KT, HD], f16, kind="ExternalInput").ap()
    wk = nc.dram_tensor("wk", [P, NKV_L, KT, HD], f16, kind="ExternalInput").ap()
    wv = nc.dram_tensor("wv", [P, KT, NKV_L * HD], f16, kind="ExternalInput").ap()
    wo = nc.dram_tensor("wo", [P, NH_L, HID], f16, kind="ExternalInput").ap()
    cosT = nc.dram_tensor("cosT", [HD, S], f16, kind="ExternalInput").ap()
    sinT = nc.dram_tensor("sinT", [HD, S], f16, kind="ExternalInput").ap()
    rmat = nc.dram_tensor("rmat", [HD, HD], f16, kind="ExternalInput").ap()
    out = nc.dram_tensor("out", [S, HID], f32, kind="ExternalOutput").ap()

    EXP = mybir.ActivationFunctionType.Exp

    with TileContext(nc) as tc:
        with (
            tc.tile_pool(name="consts", bufs=1) as consts,
            tc.tile_pool(name="qkv", bufs=1) as qkvp,
            tc.tile_pool(name="ps1", bufs=4, space="PSUM") as ps1,
            tc.tile_pool(name="ps2", bufs=2, space="PSUM") as ps2,
        ):
            ones = consts.tile([P, P], f16, tag="ones")
            nc.vector.memset(ones, 1.0)
            rot = consts.tile([HD, HD], f16, tag="rot")
            nc.sync.dma_start(out=rot, in_=rmat)

            q_sb = qkvp.tile([P, NH_L, S], f16, tag="q")
            k_sb = qkvp.tile([P, NKV_L, S], f16, tag="k")
            v_sb = qkvp.tile([P, TT, NKV_L * HD], f16, tag="v")

            # ---------------- Phase A: projections + RoPE ----------------
            with (
                tc.tile_pool(name="wqkv", bufs=1) as wp,
                tc.tile_pool(name="trig", bufs=1) as trig,
                tc.tile_pool(name="hs", bufs=2) as hsp,
                tc.tile_pool(name="ropes", bufs=4) as smalls,
            ):
                cos_sb = trig.tile([HD, S], f16, tag="cos")
                nc.sync.dma_start(out=cos_sb, in_=cosT)
                sin_sb = trig.tile([HD, S], f16, tag="sin")
                nc.sync.dma_start(out=sin_sb, in_=sinT)

                wk_sb = wp.tile([P, NKV_L, KT, HD], f16, tag="wk")
                nc.sync.dma_start(out=wk_sb, in_=wk)
                wv_sb = wp.tile([P, KT, NKV_L * HD], f16, tag="wv")
                nc.sync.dma_start(out=wv_sb, in_=wv)
                wq_sb = wp.tile([P, NH_L, KT, HD], f16, tag="wq")
                for h in range(NH_L):  # per-head DMAs so head 0 starts early
                    nc.sync.dma_start(out=wq_sb[:, h, :, :], in_=wq[:, h, :, :])

                # software pipeline: the rot-matmul + rope combine for one
                # projection is emitted while the NEXT projection's matmul
                # group runs, so PE never waits on the ScalarE copyback.
                pending = []

                def rope_flush():
                    qc, s0, dst, dsti = pending.pop(0)
                    pr = ps1.tile([P, ST], f32, tag="ps")
                    nc.tensor.matmul(pr, lhsT=rot, rhs=qc, start=True, stop=True)
                    rc = smalls.tile([P, ST], f16, tag="rc")
                    nc.scalar.copy(rc, pr)
                    t1 = smalls.tile([P, ST], f16, tag="t1")
                    nc.vector.tensor_mul(t1, qc, cos_sb[:, s0 : s0 + ST])
                    t2 = smalls.tile([P, ST], f16, tag="t2")
                    nc.vector.tensor_mul(t2, rc, sin_sb[:, s0 : s0 + ST])
                    nc.vector.tensor_add(dst[:, dsti, s0 : s0 + ST], t1, t2)

                for si in range(NSB):
                    s0 = si * ST
                    hs_blk = hsp.tile([P, KT, ST], f16, tag="hs")
                    nc.sync.dma_start(out=hs_blk, in_=hsT[:, :, s0 : s0 + ST])

                    def proj(w_slice, dst, dsti):
                        pm = ps1.tile([P, ST], f32, tag="ps")
                        for kt in range(KT):
                            nc.tensor.matmul(
                                pm,
                                lhsT=w_slice[:, kt, :],
                                rhs=hs_blk[:, kt, :],
                                start=(kt == 0),
                                stop=(kt == KT - 1),
                            )
                        qc = smalls.tile([P, ST], f16, tag="qc")
                        nc.scalar.copy(qc, pm)
                        pending.append((qc, s0, dst, dsti))

                    for j in range(NKV_L):
                        proj(wk_sb[:, j], k_sb, j)
                        if len(pending) > 1:
                            rope_flush()
                    for h in range(NH_L):
                        proj(wq_sb[:, h], q_sb, h)
                        if len(pending) > 1:
                            rope_flush()

                    # v in natural [t, d] layout: hs tiles stationary
                    for sj in range(ST // P):
                        tt = s0 // P + sj
                        pv = ps1.tile([P, NKV_L * HD], f32, tag="ps")
                        for kt in range(KT):
                            nc.tensor.matmul(
                                pv,
                                lhsT=hs_blk[:, kt, sj * P : (sj + 1) * P],
                                rhs=wv_sb[:, kt, :],
                                start=(kt == 0),
                                stop=(kt == KT - 1),
                            )
                        nc.scalar.copy(v_sb[:, tt, :], pv)
                while pending:
                    rope_flush()

            # ---------------- Phase B: attention ----------------
            with (
                tc.tile_pool(name="wo", bufs=1) as wop,
                tc.tile_pool(name="attn", bufs=1) as ap_,
                tc.tile_pool(name="pblk", bufs=2) as pp,
                tc.tile_pool(name="phalf", bufs=2) as php,
                tc.tile_pool(name="rcps", bufs=4) as rcpp,
                tc.tile_pool(name="outp", bufs=2) as op_,
            ):
                wo_sb = wop.tile([P, NH_L, HID], f16, tag="wo")
                nc.sync.dma_start(out=wo_sb, in_=wo)
                attnT = ap_.tile([P, NH_L, S], f16, tag="attnT")
                HT = TT // 2

                for h in range(NH_L):
                    j = h // GROUP
                    for si in range(NSB):
                        s0 = si * ST
                        pblk = pp.tile([P, TT, ST], f16, tag="pblk")
                        for t2_ in range(TT // 2):
                            psc = ps2.tile([P, 2, ST], f32, tag="psc")
                            for u in range(2):
                                tt = 2 * t2_ + u
                                nc.tensor.matmul(
                                    psc[:, u, :],
                                    lhsT=k_sb[:, j, tt * P : (tt + 1) * P],
                                    rhs=q_sb[:, h, s0 : s0 + ST],
                                    start=True,
                                    stop=True,
                                )
                            nc.scalar.activation(
                                out=pblk[:, 2 * t2_ : 2 * t2_ + 2, :],
                                in_=psc,
                                func=EXP,
                                scale=SCALE,
                            )
                        # softmax denominator: fp16 pairwise add halves the
                        # ones-matmul row count, PSUM f32 does the rest
                        ph = php.tile([P, HT, ST], f16, tag="ph")
                        nc.vector.tensor_add(ph, pblk[:, :HT, :], pblk[:, HT:, :])
                        pcs = ps1.tile([P, ST], f32, tag="ps")
                        for tt in range(HT):
                            nc.tensor.matmul(
                                pcs,
                                lhsT=ones,
                                rhs=ph[:, tt, :],
                                start=(tt == 0),
                                stop=(tt == HT - 1),
                            )
                        rcp = rcpp.tile([P, ST], f32, tag="rcp")
                        nc.vector.reciprocal(rcp, pcs)
                        pat = ps1.tile([P, ST], f32, tag="ps")
                        for tt in range(TT):
                            nc.tensor.matmul(
                                pat,
                                lhsT=v_sb[:, tt, j * HD : (j + 1) * HD],
                                rhs=pblk[:, tt, :],
                                start=(tt == 0),
                                stop=(tt == TT - 1),
                            )
                        nc.vector.tensor_mul(attnT[:, h, s0 : s0 + ST], pat, rcp)

                # ---------------- Phase C: o_proj ----------------
                for st in range(S // P):
                    ob = op_.tile([P, HID], f32, tag="ob")
                    for ni in range(HID // ST):
                        po = ps1.tile([P, ST], f32, tag="ps")
                        for ft in range(NH_L):
                            nc.tensor.matmul(
                                po,
                                lhsT=attnT[:, ft, st * P : (st + 1) * P],
                                rhs=wo_sb[:, ft, ni * ST : (ni + 1) * ST],
                                start=(ft == 0),
                                stop=(ft == NH_L - 1),
                            )
                        nc.vector.tensor_copy(ob[:, ni * ST : (ni + 1) * ST], po)
                    nc.sync.dma_start(out=out[st * P : (st + 1) * P, :], in_=ob)

    nc.compile()
    return nc


def _get_nc():
    if "nc" not in _CACHE:
        _CACHE["nc"] = _build()
    return _CACHE["nc"]


def kernel(hidden_states, cos, sin, Wq, Wk, Wv, Wo):
    if "/opt/trn_rl_repo" not in sys.path:
        sys.path.insert(0, "/opt/trn_rl_repo")
    from concourse.bass_utils import run_bass_kernel_spmd

    hidden_states = np.asarray(hidden_states, dtype=np.float32)
    cos = np.asarray(cos, dtype=np.float32)
    sin = np.asarray(sin, dtype=np.float32)
    Wq = np.asarray(Wq, dtype=np.float32)
    Wk = np.asarray(Wk, dtype=np.float32)
    Wv = np.asarray(Wv, dtype=np.float32)
    Wo = np.asarray(Wo, dtype=np.float32)

    nc = _get_nc()
    rm = _rot_lhsT()

    # pretiled host layouts: partition index first, contiguous per DMA slice
    def tile_khid(w):  # [HID, F] -> [P, KT, F]
        return np.ascontiguousarray(
            w.reshape(KT, P, w.shape[1]).transpose(1, 0, 2)
        ).astype(np.float16)

    in_maps = []
    hsT_b = [
        np.ascontiguousarray(hidden_states[b].T.reshape(KT, P, S).transpose(1, 0, 2))
        .astype(np.float16)
        for b in range(B)
    ]
    cosT_b = [np.ascontiguousarray(cos[b].T).astype(np.float16) for b in range(B)]
    sinT_b = [np.ascontiguousarray(sin[b].T).astype(np.float16) for b in range(B)]
    for c in range(2 * B):
        b, half = c // 2, c % 2
        fq = slice(half * NH_L * HD, (half + 1) * NH_L * HD)
        fkv = slice(half * NKV_L * HD, (half + 1) * NKV_L * HD)
        wq_t = tile_khid(Wq[:, fq]).reshape(P, KT, NH_L, HD).transpose(0, 2, 1, 3)
        wk_t = tile_khid(Wk[:, fkv]).reshape(P, KT, NKV_L, HD).transpose(0, 2, 1, 3)
        wo_t = np.ascontiguousarray(
            Wo[fq, :].reshape(NH_L, P, HID).transpose(1, 0, 2)
        ).astype(np.float16)
        in_maps.append(
            {
                "hsT": hsT_b[b],
                "wq": np.ascontiguousarray(wq_t),
                "wk": np.ascontiguousarray(wk_t),
                "wv": tile_khid(Wv[:, fkv]),
                "wo": wo_t,
                "cosT": cosT_b[b],
                "sinT": sinT_b[b],
                "rmat": rm,
            }
        )

    res = run_bass_kernel_spmd(nc, in_maps, list(range(2 * B)))
    _CACHE["last_results"] = res

    out = np.empty((B, S, HID), dtype=np.float32)
    for b in range(B):
        out[b] = res.results[2 * b]["out"] + res.results[2 * b + 1]["out"]
    return out


# revision 5
# speedup vs baseline: 1.1988x; 1.1318x over previous
"""GQA attention (RoPE + softmax + o_proj) on 8 Trainium2 NeuronCores.

Problem shapes (hardcoded): hidden_states [4, 2048, 2048], 16 q heads,
4 kv heads, head_dim 128, rope cos/sin tables given as inputs.

Sharding: core c -> (batch b = c // 2, q-head half = c % 2).  Each core
computes 8 q heads + their 2 kv heads for one batch and produces a
partial o_proj output [2048, 2048]; the host sums the two halves per
batch (tensor parallel, no device collectives).

All matmuls run in fp16 (1 cycle/row on PE) with fp32 PSUM accumulation:
  - q^T/k^T = W^T @ hs^T with hid on partitions (weights are natural lhsT)
  - RoPE via a +-1 permutation matmul (rotate_half), software-pipelined
    one projection behind so PE never waits on the PSUM copyback
  - scores^T[t, s] with k^T tiles stationary; exp via ScalarE (fused
    1/sqrt(d) scale) reads two PSUM banks per instruction and writes
    P^T fp16 straight to SBUF; the exp-dependent stages of iteration i
    are emitted after the score matmuls of iteration i+1 (software
    pipeline) so PE never idles waiting for ScalarE
  - softmax denominators: two fp16 DVE pairwise adds over P^T quarters,
    then an all-ones stationary matmul (result replicated across
    partitions = pre-broadcast), fast DVE reciprocal, fused
    normalize+cast on the attn PSUM->SBUF copyback
  - attn^T[d, s] = v-tiles stationary @ P^T; o_proj with attn^T tiles
    stationary over Wo, interleaved per s-block right after the heads
    of that block finish.
"""

import sys

import numpy as np

B, S, HID = 4, 2048, 2048
NH, NKV, HD = 16, 4, 128
NH_L = 8        # q heads per core
NKV_L = 2       # kv heads per core
GROUP = NH // NKV
P = 128
ST = 512        # s-block (matmul free dim)
NSB = S // ST   # 4 s-blocks
KT = HID // P   # 16 contraction tiles over hidden
TT = S // P     # 16 key/t tiles
SCALE = 1.0 / float(np.sqrt(HD))

_CACHE = {}


def _rot_lhsT():
    """Stationary operand R such that R.T @ q^T = rotate_half(q)^T."""
    r = np.zeros((HD, HD), dtype=np.float16)
    half = HD // 2
    i = np.arange(half)
    r[i + half, i] = -1.0
    r[i, i + half] = 1.0
    return r


def _build():
    if "/opt/trn_rl_repo" not in sys.path:
        sys.path.insert(0, "/opt/trn_rl_repo")
    import concourse.mybir as mybir
    from concourse import bacc
    from concourse.tile import TileContext

    dt = mybir.dt
    f16, f32 = dt.float16, dt.float32

    nc = bacc.Bacc("TRN2", target_bir_lowering=False, debug=False, num_devices=8)
    # host-pretiled layouts (see kernel() below)
    hsT = nc.dram_tensor("hsT", [P, KT, S], f16, kind="ExternalInput").ap()
    wq = nc.dram_tensor("wq", [P, NH_L, KT, HD], f16, kind="ExternalInput").ap()
    wk = nc.dram_tensor("wk", [P, NKV_L, KT, HD], f16, kind="ExternalInput").ap()
    wv = nc.dram_tensor("wv", [P, KT, NKV_L * HD], f16, kind="ExternalInput").ap()
    wo = nc.dram_tensor("wo", [P, NH_L, HID], f16, kind="ExternalInput").ap()
    cosT = nc.dram_tensor("cosT", [HD, S], f16, kind="ExternalInput").ap()
    sinT = nc.dram_tensor("sinT", [HD, S], f16, kind="ExternalInput").ap()
    rmat = nc.dram_tensor("rmat", [HD, HD], f16, kind="ExternalInput").ap()
    out = nc.dram_tensor("out", [S, HID], f32, kind="ExternalOutput").ap()

    EXP = mybir.ActivationFunctionType.Exp

    with TileContext(nc) as tc:
        with (
            tc.tile_pool(name="consts", bufs=1) as consts,
            tc.tile_pool(name="qkv", bufs=1) as qkvp,
        ):
            ones = consts.tile([P, P], f16, tag="ones")
            nc.vector.memset(ones, 1.0)
            rot = consts.tile([HD, HD], f16, tag="rot")
            nc.gpsimd.dma_start(out=rot, in_=rmat)

            q_sb = qkvp.tile([P, NH_L, S], f16, tag="q")
            k_sb = qkvp.tile([P, NKV_L, S], f16, tag="k")
            v_sb = qkvp.tile([P, TT, NKV_L * HD], f16, tag="v")

            # ---------------- Phase A: projections + RoPE ----------------
            with (
                tc.tile_pool(name="wqkv", bufs=1) as wp,
                tc.tile_pool(name="trig", bufs=1) as trig,
                tc.tile_pool(name="hs", bufs=2) as hsp,
                tc.tile_pool(name="ropes", bufs=4) as smalls,
                tc.tile_pool(name="psA", bufs=6, space="PSUM") as psA,
            ):
                # hs block 0 first (its consumers are the head of the program)
                hs_blks = {}
                hs_first = hsp.tile([P, KT, ST], f16, tag="hs")
                nc.scalar.dma_start(out=hs_first, in_=hsT[:, :, 0:ST])
                hs_blks[0] = hs_first

                cos_sb = trig.tile([HD, S], f16, tag="cos")
                nc.gpsimd.dma_start(out=cos_sb, in_=cosT)
                sin_sb = trig.tile([HD, S], f16, tag="sin")
                nc.gpsimd.dma_start(out=sin_sb, in_=sinT)

                wk_sb = wp.tile([P, NKV_L, KT, HD], f16, tag="wk")
                nc.sync.dma_start(out=wk_sb, in_=wk)
                wv_sb = wp.tile([P, KT, NKV_L * HD], f16, tag="wv")
                nc.sync.dma_start(out=wv_sb, in_=wv)
                wq_sb = wp.tile([P, NH_L, KT, HD], f16, tag="wq")
                for h in range(NH_L):  # per-head DMAs so head 0 starts early
                    nc.sync.dma_start(out=wq_sb[:, h, :, :], in_=wq[:, h, :, :])

                # software pipeline: the rot-matmul + rope combine for one
                # projection is emitted while the NEXT projection's matmul
                # group runs, so PE never waits on the PSUM copyback.
                pending = []

                def rope_flush():
                    qc, s0, dst, dsti = pending.pop(0)
                    pr = psA.tile([P, ST], f32, tag="ps")
                    nc.tensor.matmul(pr, lhsT=rot, rhs=qc, start=True, stop=True)
                    rc = smalls.tile([P, ST], f16, tag="rc")
                    nc.vector.tensor_copy(rc, pr)
                    t1 = smalls.tile([P, ST], f16, tag="t1")
                    nc.vector.tensor_mul(t1, qc, cos_sb[:, s0 : s0 + ST])
                    t2 = smalls.tile([P, ST], f16, tag="t2")
                    nc.vector.tensor_mul(t2, rc, sin_sb[:, s0 : s0 + ST])
                    nc.vector.tensor_add(dst[:, dsti, s0 : s0 + ST], t1, t2)

                for si in range(NSB):
                    s0 = si * ST
                    if si in hs_blks:
                        hs_blk = hs_blks[si]
                    else:
                        hs_blk = hsp.tile([P, KT, ST], f16, tag="hs")
                        nc.scalar.dma_start(out=hs_blk, in_=hsT[:, :, s0 : s0 + ST])

                    def proj(w_slice, dst, dsti):
                        pm = psA.tile([P, ST], f32, tag="ps")
                        for kt in range(KT):
                            nc.tensor.matmul(
                                pm,
                                lhsT=w_slice[:, kt, :],
                                rhs=hs_blk[:, kt, :],
                                start=(kt == 0),
                                stop=(kt == KT - 1),
                            )
                        qc = smalls.tile([P, ST], f16, tag="qc")
                        nc.vector.tensor_copy(qc, pm)
                        pending.append((qc, s0, dst, dsti))

                    for j in range(NKV_L):
                        proj(wk_sb[:, j], k_sb, j)
                        if len(pending) > 1:
                            rope_flush()
                    for h in range(NH_L):
                        proj(wq_sb[:, h], q_sb, h)
                        if len(pending) > 1:
                            rope_flush()

                    # v in natural [t, d] layout: hs tiles stationary
                    for sj in range(ST // P):
                        tt = s0 // P + sj
                        pv = psA.tile([P, NKV_L * HD], f32, tag="ps")
                        for kt in range(KT):
                            nc.tensor.matmul(
                                pv,
                                lhsT=hs_blk[:, kt, sj * P : (sj + 1) * P],
                                rhs=wv_sb[:, kt, :],
                                start=(kt == 0),
                                stop=(kt == KT - 1),
                            )
                        nc.scalar.copy(v_sb[:, tt, :], pv)
                while pending:
                    rope_flush()

            # ---------------- Phase B: attention + interleaved o_proj ------
            with (
                tc.tile_pool(name="wo", bufs=1) as wop,
                tc.tile_pool(name="attn", bufs=1) as ap_,
                tc.tile_pool(name="pblk", bufs=2) as pp,
                tc.tile_pool(name="phalf", bufs=2) as php,
                tc.tile_pool(name="rcps", bufs=4) as rcpp,
                tc.tile_pool(name="outp", bufs=2) as op_,
                tc.tile_pool(name="psB", bufs=4, space="PSUM") as psB,
                tc.tile_pool(name="psc", bufs=2, space="PSUM") as pscp,
            ):
                wo_sb = wop.tile([P, NH_L, HID], f16, tag="wo")
                nc.sync.dma_start(out=wo_sb, in_=wo)
                attnT = ap_.tile([P, NH_L, S], f16, tag="attnT")
                QT = TT // 4

                def scores(h, si, pblk):
                    j = h // GROUP
                    s0 = si * ST
                    for t2_ in range(TT // 2):
                        psc = pscp.tile([P, 2, ST], f32, tag="psc")
                        for u in range(2):
                            tt = 2 * t2_ + u
                            nc.tensor.matmul(
                                psc[:, u, :],
                                lhsT=k_sb[:, j, tt * P : (tt + 1) * P],
                                rhs=q_sb[:, h, s0 : s0 + ST],
                                start=True,
                                stop=True,
                            )
                        nc.scalar.activation(
                            out=pblk[:, 2 * t2_ : 2 * t2_ + 2, :],
                            in_=psc,
                            func=EXP,
                            scale=SCALE,
                        )

                def post(h, si, pblk):
                    j = h // GROUP
                    s0 = si * ST
                    # two fp16 tree levels -> 4 tiles for the ones-matmul
                    ph = php.tile([P, TT // 2, ST], f16, tag="ph")
                    nc.vector.tensor_add(
                        ph, pblk[:, : TT // 2, :], pblk[:, TT // 2 :, :]
                    )
                    nc.vector.tensor_add(
                        ph[:, :QT, :], ph[:, :QT, :], ph[:, QT : 2 * QT, :]
                    )
                    pcs = psB.tile([P, ST], f32, tag="ps")
                    for tt in range(QT):
                        nc.tensor.matmul(
                            pcs,
                            lhsT=ones,
                            rhs=ph[:, tt, :],
                            start=(tt == 0),
                            stop=(tt == QT - 1),
                        )
                    rcp = rcpp.tile([P, ST], f32, tag="rcp")
                    nc.vector.reciprocal_approx_fast(out=rcp, in_=pcs)
                    pat = psB.tile([P, ST], f32, tag="ps")
                    for tt in range(TT):
                        nc.tensor.matmul(
                            pat,
                            lhsT=v_sb[:, tt, j * HD : (j + 1) * HD],
                            rhs=pblk[:, tt, :],
                            start=(tt == 0),
                            stop=(tt == TT - 1),
                        )
                    nc.vector.tensor_mul(attnT[:, h, s0 : s0 + ST], pat, rcp)

                def o_proj(si):
                    for sj in range(ST // P):
                        st = si * (ST // P) + sj
                        ob = op_.tile([P, HID], f32, tag="ob")
                        for ni in range(HID // ST):
                            po = psB.tile([P, ST], f32, tag="ps")
                            for ft in range(NH_L):
                                nc.tensor.matmul(
                                    po,
                                    lhsT=attnT[:, ft, st * P : (st + 1) * P],
                                    rhs=wo_sb[:, ft, ni * ST : (ni + 1) * ST],
                                    start=(ft == 0),
                                    stop=(ft == NH_L - 1),
                                )
                            nc.scalar.copy(ob[:, ni * ST : (ni + 1) * ST], po)
                        nc.sync.dma_start(out=out[st * P : (st + 1) * P, :], in_=ob)

                prev = None
                for si in range(NSB):
                    for h in range(NH_L):
                        pblk = pp.tile([P, TT, ST], f16, tag="pblk")
                        scores(h, si, pblk)
                        if prev is not None:
                            post(*prev)
                        prev = (h, si, pblk)
                    post(*prev)
                    prev = None
                    o_proj(si)

    nc.compile()
    return nc


def _get_nc():
    if "nc" not in _CACHE:
        _CACHE["nc"] = _build()
    return _CACHE["nc"]


def kernel(hidden_states, cos, sin, Wq, Wk, Wv, Wo):
    if "/opt/trn_rl_repo" not in sys.path:
        sys.path.insert(0, "/opt/trn_rl_repo")
    from concourse.bass_utils import run_bass_kernel_spmd

    hidden_states = np.asarray(hidden_states, dtype=np.float32)
    cos = np.asarray(cos, dtype=np.float32)
    sin = np.asarray(sin, dtype=np.float32)
    Wq = np.asarray(Wq, dtype=np.float32)
    Wk = np.asarray(Wk, dtype=np.float32)
    Wv = np.asarray(Wv, dtype=np.float32)
    Wo = np.asarray(Wo, dtype=np.float32)

    nc = _get_nc()
    rm = _rot_lhsT()

    # pretiled host layouts: partition index first, contiguous per DMA slice
    def tile_khid(w):  # [HID, F] -> [P, KT, F]
        return np.ascontiguousarray(
            w.reshape(KT, P, w.shape[1]).transpose(1, 0, 2)
        ).astype(np.float16)

    in_maps = []
    hsT_b = [
        np.ascontiguousarray(hidden_states[b].T.reshape(KT, P, S).transpose(1, 0, 2))
        .astype(np.float16)
        for b in range(B)
    ]
    cosT_b = [np.ascontiguousarray(cos[b].T).astype(np.float16) for b in range(B)]
    sinT_b = [np.ascontiguousarray(sin[b].T).astype(np.float16) for b in range(B)]
    for c in range(2 * B):
        b, half = c // 2, c % 2
        fq = slice(half * NH_L * HD, (half + 1) * NH_L * HD)
        fkv = slice(half * NKV_L * HD, (half + 1) * NKV_L * HD)
        wq_t = tile_khid(Wq[:, fq]).reshape(P, KT, NH_L, HD).transpose(0, 2, 1, 3)
        wk_t = tile_khid(Wk[:, fkv]).reshape(P, KT, NKV_L, HD).transpose(0, 2, 1, 3)
        wo_t = np.ascontiguousarray(
            Wo[fq, :].reshape(NH_L, P, HID).transpose(1, 0, 2)
        ).astype(np.float16)
        in_maps.append(
            {
                "hsT": hsT_b[b],
                "wq": np.ascontiguousarray(wq_t),
                "wk": np.ascontiguousarray(wk_t),
                "wv": tile_khid(Wv[:, fkv]),
                "wo": wo_t,
                "cosT": cosT_b[b],
                "sinT": sinT_b[b],
                "rmat": rm,
            }
        )

    res = run_bass_kernel_spmd(nc, in_maps, list(range(2 * B)))
    _CACHE["last_results"] = res

    out = np.empty((B, S, HID), dtype=np.float32)
    for b in range(B):
        out[b] = res.results[2 * b]["out"] + res.results[2 * b + 1]["out"]
    return out


# revision 8
# speedup vs baseline: 1.2898x; 1.0759x over previous
"""GQA attention (RoPE + softmax + o_proj) on 8 Trainium2 NeuronCores.

Problem shapes (hardcoded): hidden_states [4, 2048, 2048], 16 q heads,
4 kv heads, head_dim 128, rope cos/sin tables given as inputs.

Sharding: core c -> (batch b = c // 2, q-head half = c % 2).  Each core
computes 8 q heads + their 2 kv heads for one batch and produces a
partial o_proj output [2048, 2048]; the host sums the two halves per
batch (tensor parallel, no device collectives).

All matmuls run in fp16 (1 cycle/row on PE) with fp32 PSUM accumulation:
  - q^T/k^T = W^T @ hs^T with hid on partitions (weights are natural lhsT)
  - RoPE via a +-1 permutation matmul (rotate_half), software-pipelined
    one projection behind so PE never waits on the PSUM copyback
  - scores^T[t, s] with k^T tiles stationary; exp via ScalarE (fused
    1/sqrt(d) scale) reads two PSUM banks per instruction and writes
    P^T fp16 straight to SBUF; the exp-dependent stages of iteration i
    are emitted after the score matmuls of iteration i+1 (software
    pipeline) so PE never idles waiting for ScalarE
  - softmax denominators: two fp16 DVE pairwise adds over P^T quarters,
    then an all-ones stationary matmul (result replicated across
    partitions = pre-broadcast), fast DVE reciprocal, fused
    normalize+cast on the attn PSUM->SBUF copyback
  - attn^T[d, s] = v-tiles stationary @ P^T; o_proj with attn^T tiles
    stationary over Wo, interleaved per s-block right after the heads
    of that block finish.
"""

import sys

import numpy as np

B, S, HID = 4, 2048, 2048
NH, NKV, HD = 16, 4, 128
NH_L = 8        # q heads per core
NKV_L = 2       # kv heads per core
GROUP = NH // NKV
P = 128
ST = 512        # s-block (matmul free dim)
NSB = S // ST   # 4 s-blocks
KT = HID // P   # 16 contraction tiles over hidden
TT = S // P     # 16 key/t tiles
SCALE = 1.0 / float(np.sqrt(HD))

_CACHE = {}


def _rot_lhsT():
    """Stationary operand R such that R.T @ q^T = rotate_half(q)^T."""
    r = np.zeros((HD, HD), dtype=np.float16)
    half = HD // 2
    i = np.arange(half)
    r[i + half, i] = -1.0
    r[i, i + half] = 1.0
    return r


def _build():
    if "/opt/trn_rl_repo" not in sys.path:
        sys.path.insert(0, "/opt/trn_rl_repo")
    import concourse.mybir as mybir
    from concourse import bacc
    from concourse.tile import TileContext

    dt = mybir.dt
    f16, f32 = dt.float16, dt.float32

    nc = bacc.Bacc("TRN2", target_bir_lowering=False, debug=False, num_devices=8)
    # host-pretiled layouts (see kernel() below)
    hsT = nc.dram_tensor("hsT", [P, NSB, KT, ST], f16, kind="ExternalInput").ap()
    wq = nc.dram_tensor("wq", [P, NH_L, KT, HD], f16, kind="ExternalInput").ap()
    wk = nc.dram_tensor("wk", [P, NKV_L, KT, HD], f16, kind="ExternalInput").ap()
    wv = nc.dram_tensor("wv", [P, KT, NKV_L * HD], f16, kind="ExternalInput").ap()
    wo = nc.dram_tensor("wo", [P, NH_L, HID], f16, kind="ExternalInput").ap()
    cosT = nc.dram_tensor("cosT", [HD, S], f16, kind="ExternalInput").ap()
    sinT = nc.dram_tensor("sinT", [HD, S], f16, kind="ExternalInput").ap()
    rmat = nc.dram_tensor("rmat", [HD, HD], f16, kind="ExternalInput").ap()
    out = nc.dram_tensor("out", [S, HID], f32, kind="ExternalOutput").ap()

    EXP = mybir.ActivationFunctionType.Exp

    with TileContext(nc) as tc:
        with (
            tc.tile_pool(name="consts", bufs=1) as consts,
            tc.tile_pool(name="qkv", bufs=1) as qkvp,
        ):
            ones = consts.tile([P, P], f16, tag="ones")
            nc.vector.memset(ones, 1.0)
            rot = consts.tile([HD, HD], f16, tag="rot")
            nc.gpsimd.dma_start(out=rot, in_=rmat)

            q_sb = qkvp.tile([P, NH_L, S], f16, tag="q")
            k_sb = qkvp.tile([P, NKV_L, S], f16, tag="k")
            v_sb = qkvp.tile([P, TT, NKV_L * HD], f16, tag="v")

            # ---------------- Phase A: projections + RoPE ----------------
            with (
                tc.tile_pool(name="wqkv", bufs=1) as wp,
                tc.tile_pool(name="trig", bufs=1) as trig,
                tc.tile_pool(name="hs", bufs=2) as hsp,
                tc.tile_pool(name="ropes", bufs=4) as smalls,
                tc.tile_pool(name="psA", bufs=6, space="PSUM") as psA,
            ):
                # hs block 0 first (its consumers are the head of the program)
                hs_blks = {}
                hs_first = hsp.tile([P, KT, ST], f16, tag="hs")
                nc.scalar.dma_start(out=hs_first, in_=hsT[:, 0, :, :])
                hs_blks[0] = hs_first

                cos_sb = trig.tile([HD, S], f16, tag="cos")
                nc.gpsimd.dma_start(out=cos_sb, in_=cosT)
                sin_sb = trig.tile([HD, S], f16, tag="sin")
                nc.gpsimd.dma_start(out=sin_sb, in_=sinT)

                wk_sb = wp.tile([P, NKV_L, KT, HD], f16, tag="wk")
                nc.sync.dma_start(out=wk_sb, in_=wk)
                wv_sb = wp.tile([P, KT, NKV_L * HD], f16, tag="wv")
                nc.sync.dma_start(out=wv_sb, in_=wv)
                wq_sb = wp.tile([P, NH_L, KT, HD], f16, tag="wq")
                for h in range(NH_L):  # per-head DMAs so head 0 starts early
                    nc.sync.dma_start(out=wq_sb[:, h, :, :], in_=wq[:, h, :, :])

                # software pipeline: the rot-matmul + rope combine for one
                # projection is emitted while the NEXT projection's matmul
                # group runs, so PE never waits on the PSUM copyback.
                pending = []

                def rope_flush():
                    qc, s0, dst, dsti = pending.pop(0)
                    pr = psA.tile([P, ST], f32, tag="ps")
                    nc.tensor.matmul(pr, lhsT=rot, rhs=qc, start=True, stop=True)
                    rc = smalls.tile([P, ST], f16, tag="rc")
                    nc.vector.tensor_copy(rc, pr)
                    t1 = smalls.tile([P, ST], f16, tag="t1")
                    nc.vector.tensor_mul(t1, qc, cos_sb[:, s0 : s0 + ST])
                    t2 = smalls.tile([P, ST], f16, tag="t2")
                    nc.vector.tensor_mul(t2, rc, sin_sb[:, s0 : s0 + ST])
                    nc.vector.tensor_add(dst[:, dsti, s0 : s0 + ST], t1, t2)

                for si in range(NSB):
                    s0 = si * ST
                    if si in hs_blks:
                        hs_blk = hs_blks[si]
                    else:
                        hs_blk = hsp.tile([P, KT, ST], f16, tag="hs")
                        nc.scalar.dma_start(out=hs_blk, in_=hsT[:, si, :, :])

                    def proj(w_slice, dst, dsti):
                        pm = psA.tile([P, ST], f32, tag="ps")
                        for kt in range(KT):
                            nc.tensor.matmul(
                                pm,
                                lhsT=w_slice[:, kt, :],
                                rhs=hs_blk[:, kt, :],
                                start=(kt == 0),
                                stop=(kt == KT - 1),
                            )
                        qc = smalls.tile([P, ST], f16, tag="qc")
                        nc.vector.tensor_copy(qc, pm)
                        pending.append((qc, s0, dst, dsti))

                    # v first: needs only hs + the small wv
                    for sj in range(ST // P):
                        tt = s0 // P + sj
                        pv = psA.tile([P, NKV_L * HD], f32, tag="ps")
                        for kt in range(KT):
                            nc.tensor.matmul(
                                pv,
                                lhsT=hs_blk[:, kt, sj * P : (sj + 1) * P],
                                rhs=wv_sb[:, kt, :],
                                start=(kt == 0),
                                stop=(kt == KT - 1),
                            )
                        nc.scalar.copy(v_sb[:, tt, :], pv)
                    for j in range(NKV_L):
                        proj(wk_sb[:, j], k_sb, j)
                        if len(pending) > 1:
                            rope_flush()
                    for h in range(NH_L):
                        proj(wq_sb[:, h], q_sb, h)
                        if len(pending) > 1:
                            rope_flush()
                while pending:
                    rope_flush()

            # ---------------- Phase B: attention + interleaved o_proj ------
            with (
                tc.tile_pool(name="wo", bufs=1) as wop,
                tc.tile_pool(name="attn", bufs=1) as ap_,
                tc.tile_pool(name="pblk", bufs=2) as pp,
                tc.tile_pool(name="phalf", bufs=2) as php,
                tc.tile_pool(name="rcps", bufs=4) as rcpp,
                tc.tile_pool(name="outp", bufs=2) as op_,
                tc.tile_pool(name="psB", bufs=4, space="PSUM") as psB,
                tc.tile_pool(name="psc", bufs=2, space="PSUM") as pscp,
            ):
                wo_sb = wop.tile([P, NH_L, HID], f16, tag="wo")
                nc.sync.dma_start(out=wo_sb, in_=wo)
                attnT = ap_.tile([P, NH_L, S], f16, tag="attnT")
                QT = TT // 4

                def scores(h, si, pblk):
                    j = h // GROUP
                    s0 = si * ST
                    for t2_ in range(TT // 2):
                        psc = pscp.tile([P, 2, ST], f32, tag="psc")
                        for u in range(2):
                            tt = 2 * t2_ + u
                            nc.tensor.matmul(
                                psc[:, u, :],
                                lhsT=k_sb[:, j, tt * P : (tt + 1) * P],
                                rhs=q_sb[:, h, s0 : s0 + ST],
                                start=True,
                                stop=True,
                            )
                        nc.scalar.activation(
                            out=pblk[:, 2 * t2_ : 2 * t2_ + 2, :],
                            in_=psc,
                            func=EXP,
                            scale=SCALE,
                        )

                def post(h, si, pblk):
                    j = h // GROUP
                    s0 = si * ST
                    # two fp16 tree levels -> 4 tiles for the ones-matmul
                    ph = php.tile([P, TT // 2, ST], f16, tag="ph")
                    nc.vector.tensor_add(
                        ph, pblk[:, : TT // 2, :], pblk[:, TT // 2 :, :]
                    )
                    nc.vector.tensor_add(
                        ph[:, :QT, :], ph[:, :QT, :], ph[:, QT : 2 * QT, :]
                    )
                    pat = psB.tile([P, ST], f32, tag="ps")
                    for tt in range(TT):
                        nc.tensor.matmul(
                            pat,
                            lhsT=v_sb[:, tt, j * HD : (j + 1) * HD],
                            rhs=pblk[:, tt, :],
                            start=(tt == 0),
                            stop=(tt == TT - 1),
                        )
                    pcs = psB.tile([P, ST], f32, tag="ps")
                    for tt in range(QT):
                        nc.tensor.matmul(
                            pcs,
                            lhsT=ones,
                            rhs=ph[:, tt, :],
                            start=(tt == 0),
                            stop=(tt == QT - 1),
                        )
                    rcp = rcpp.tile([P, ST], f32, tag="rcp")
                    nc.vector.reciprocal_approx_fast(out=rcp, in_=pcs)
                    nc.vector.tensor_mul(attnT[:, h, s0 : s0 + ST], pat, rcp)

                def o_proj(si):
                    for sj in range(ST // P):
                        st = si * (ST // P) + sj
                        ob = op_.tile([P, HID], f32, tag="ob")
                        for ni in range(HID // ST):
                            po = psB.tile([P, ST], f32, tag="ps")
                            for ft in range(NH_L):
                                nc.tensor.matmul(
                                    po,
                                    lhsT=attnT[:, ft, st * P : (st + 1) * P],
                                    rhs=wo_sb[:, ft, ni * ST : (ni + 1) * ST],
                                    start=(ft == 0),
                                    stop=(ft == NH_L - 1),
                                )
                            nc.scalar.copy(ob[:, ni * ST : (ni + 1) * ST], po)
                        nc.sync.dma_start(out=out[st * P : (st + 1) * P, :], in_=ob)

                prev = None
                for si in range(NSB):
                    for h in range(NH_L):
                        pblk = pp.tile([P, TT, ST], f16, tag="pblk")
                        scores(h, si, pblk)
                        if prev is not None:
                            post(*prev)
                        prev = (h, si, pblk)
                    post(*prev)
                    prev = None
                    o_proj(si)

    nc.compile()
    return nc


def _get_nc():
    if "nc" not in _CACHE:
        _CACHE["nc"] = _build()
    return _CACHE["nc"]


def kernel(hidden_states, cos, sin, Wq, Wk, Wv, Wo):
    if "/opt/trn_rl_repo" not in sys.path:
        sys.path.insert(0, "/opt/trn_rl_repo")
    from concourse.bass_utils import run_bass_kernel_spmd

    hidden_states = np.asarray(hidden_states, dtype=np.float32)
    cos = np.asarray(cos, dtype=np.float32)
    sin = np.asarray(sin, dtype=np.float32)
    Wq = np.asarray(Wq, dtype=np.float32)
    Wk = np.asarray(Wk, dtype=np.float32)
    Wv = np.asarray(Wv, dtype=np.float32)
    Wo = np.asarray(Wo, dtype=np.float32)

    nc = _get_nc()
    rm = _rot_lhsT()

    # pretiled host layouts: partition index first, contiguous per DMA slice
    def tile_khid(w):  # [HID, F] -> [P, KT, F]
        return np.ascontiguousarray(
            w.reshape(KT, P, w.shape[1]).transpose(1, 0, 2)
        ).astype(np.float16)

    in_maps = []
    hsT_b = [
        np.ascontiguousarray(
            hidden_states[b].T.reshape(KT, P, NSB, ST).transpose(1, 2, 0, 3)
        ).astype(np.float16)
        for b in range(B)
    ]
    cosT_b = [np.ascontiguousarray(cos[b].T).astype(np.float16) for b in range(B)]
    sinT_b = [np.ascontiguousarray(sin[b].T).astype(np.float16) for b in range(B)]
    for c in range(2 * B):
        b, half = c // 2, c % 2
        fq = slice(half * NH_L * HD, (half + 1) * NH_L * HD)
        fkv = slice(half * NKV_L * HD, (half + 1) * NKV_L * HD)
        wq_t = tile_khid(Wq[:, fq]).reshape(P, KT, NH_L, HD).transpose(0, 2, 1, 3)
        wk_t = tile_khid(Wk[:, fkv]).reshape(P, KT, NKV_L, HD).transpose(0, 2, 1, 3)
        wo_t = np.ascontiguousarray(
            Wo[fq, :].reshape(NH_L, P, HID).transpose(1, 0, 2)
        ).astype(np.float16)
        in_maps.append(
            {
                "hsT": hsT_b[b],
                "wq": np.ascontiguousarray(wq_t),
                "wk": np.ascontiguousarray(wk_t),
                "wv": tile_khid(Wv[:, fkv]),
                "wo": wo_t,
                "cosT": cosT_b[b],
                "sinT": sinT_b[b],
                "rmat": rm,
            }
        )

    res = run_bass_kernel_spmd(nc, in_maps, list(range(2 * B)))
    _CACHE["last_results"] = res

    out = np.empty((B, S, HID), dtype=np.float32)
    for b in range(B):
        out[b] = res.results[2 * b]["out"] + res.results[2 * b + 1]["out"]
    return out


# revision 9
# speedup vs baseline: 1.2935x; 1.0029x over previous
"""GQA attention (RoPE + softmax + o_proj) on 8 Trainium2 NeuronCores.

Problem shapes (hardcoded): hidden_states [4, 2048, 2048], 16 q heads,
4 kv heads, head_dim 128, rope cos/sin tables given as inputs.

Sharding: core c -> (batch b = c // 2, q-head half = c % 2).  Each core
computes 8 q heads + their 2 kv heads for one batch and produces a
partial o_proj output [2048, 2048]; the host sums the two halves per
batch (tensor parallel, no device collectives).

All matmuls run in fp16 (1 cycle/row on PE) with fp32 PSUM accumulation:
  - q^T/k^T = W^T @ hs^T with hid on partitions (weights are natural lhsT)
  - RoPE via a +-1 permutation matmul (rotate_half), software-pipelined
    one projection behind so PE never waits on the PSUM copyback
  - scores^T[t, s] with k^T tiles stationary; exp via ScalarE (fused
    1/sqrt(d) scale) reads two PSUM banks per instruction and writes
    P^T fp16 straight to SBUF; the exp-dependent stages of iteration i
    are emitted after the score matmuls of iteration i+1 (software
    pipeline) so PE never idles waiting for ScalarE
  - softmax denominators: two fp16 DVE pairwise adds over P^T quarters,
    then an all-ones stationary matmul (result replicated across
    partitions = pre-broadcast), fast DVE reciprocal, fused
    normalize+cast on the attn PSUM->SBUF copyback
  - attn^T[d, s] = v-tiles stationary @ P^T; o_proj with attn^T tiles
    stationary over Wo, interleaved per s-block right after the heads
    of that block finish.
"""

import sys

import numpy as np

B, S, HID = 4, 2048, 2048
NH, NKV, HD = 16, 4, 128
NH_L = 8        # q heads per core
NKV_L = 2       # kv heads per core
GROUP = NH // NKV
P = 128
ST = 512        # s-block (matmul free dim)
NSB = S // ST   # 4 s-blocks
KT = HID // P   # 16 contraction tiles over hidden
TT = S // P     # 16 key/t tiles
SCALE = 1.0 / float(np.sqrt(HD))

_CACHE = {}


def _rot_lhsT():
    """Stationary operand R such that R.T @ q^T = rotate_half(q)^T."""
    r = np.zeros((HD, HD), dtype=np.float16)
    half = HD // 2
    i = np.arange(half)
    r[i + half, i] = -1.0
    r[i, i + half] = 1.0
    return r


def _build():
    if "/opt/trn_rl_repo" not in sys.path:
        sys.path.insert(0, "/opt/trn_rl_repo")
    import concourse.mybir as mybir
    from concourse import bacc
    from concourse.tile import TileContext

    dt = mybir.dt
    f16, f32 = dt.float16, dt.float32

    nc = bacc.Bacc("TRN2", target_bir_lowering=False, debug=False, num_devices=8)
    # host-pretiled layouts (see kernel() below)
    hsT = nc.dram_tensor("hsT", [P, NSB, KT, ST], f16, kind="ExternalInput").ap()
    wq = nc.dram_tensor("wq", [P, NH_L, KT, HD], f16, kind="ExternalInput").ap()
    wk = nc.dram_tensor("wk", [P, NKV_L, KT, HD], f16, kind="ExternalInput").ap()
    wv = nc.dram_tensor("wv", [P, KT, NKV_L * HD], f16, kind="ExternalInput").ap()
    wo = nc.dram_tensor("wo", [P, NH_L, HID], f16, kind="ExternalInput").ap()
    cosT = nc.dram_tensor("cosT", [HD, S], f16, kind="ExternalInput").ap()
    sinT = nc.dram_tensor("sinT", [HD, S], f16, kind="ExternalInput").ap()
    rmat = nc.dram_tensor("rmat", [HD, HD], f16, kind="ExternalInput").ap()
    out = nc.dram_tensor("out", [S, HID], f32, kind="ExternalOutput").ap()

    EXP = mybir.ActivationFunctionType.Exp

    with TileContext(nc) as tc:
        with (
            tc.tile_pool(name="consts", bufs=1) as consts,
            tc.tile_pool(name="qkv", bufs=1) as qkvp,
        ):
            ones = consts.tile([P, P], f16, tag="ones")
            nc.vector.memset(ones, 1.0)
            rot = consts.tile([HD, HD], f16, tag="rot")
            nc.gpsimd.dma_start(out=rot, in_=rmat)

            q_sb = qkvp.tile([P, NH_L, S], f16, tag="q")
            k_sb = qkvp.tile([P, NKV_L, S], f16, tag="k")
            v_sb = qkvp.tile([P, TT, NKV_L * HD], f16, tag="v")

            # ---------------- Phase A: projections + RoPE ----------------
            with (
                tc.tile_pool(name="wqkv", bufs=1) as wp,
                tc.tile_pool(name="trig", bufs=1) as trig,
                tc.tile_pool(name="hs", bufs=2) as hsp,
                tc.tile_pool(name="ropes", bufs=4) as smalls,
                tc.tile_pool(name="psA", bufs=6, space="PSUM") as psA,
            ):
                # hs block 0 first (its consumers are the head of the program)
                hs_blks = {}
                hs_first = hsp.tile([P, KT, ST], f16, tag="hs")
                nc.scalar.dma_start(out=hs_first, in_=hsT[:, 0, :, :])
                hs_blks[0] = hs_first

                wv_sb = wp.tile([P, KT, NKV_L * HD], f16, tag="wv")
                nc.gpsimd.dma_start(out=wv_sb, in_=wv)
                cos_sb = trig.tile([HD, S], f16, tag="cos")
                nc.gpsimd.dma_start(out=cos_sb, in_=cosT)
                sin_sb = trig.tile([HD, S], f16, tag="sin")
                nc.gpsimd.dma_start(out=sin_sb, in_=sinT)

                wk_sb = wp.tile([P, NKV_L, KT, HD], f16, tag="wk")
                nc.sync.dma_start(out=wk_sb, in_=wk)
                wq_sb = wp.tile([P, NH_L, KT, HD], f16, tag="wq")
                for h in range(NH_L):  # per-head DMAs so head 0 starts early
                    nc.sync.dma_start(out=wq_sb[:, h, :, :], in_=wq[:, h, :, :])

                # software pipeline: the rot-matmul + rope combine for one
                # projection is emitted while the NEXT projection's matmul
                # group runs, so PE never waits on the PSUM copyback.
                pending = []

                def rope_flush():
                    qc, s0, dst, dsti = pending.pop(0)
                    pr = psA.tile([P, ST], f32, tag="ps")
                    nc.tensor.matmul(pr, lhsT=rot, rhs=qc, start=True, stop=True)
                    rc = smalls.tile([P, ST], f16, tag="rc")
                    nc.vector.tensor_copy(rc, pr)
                    t1 = smalls.tile([P, ST], f16, tag="t1")
                    nc.vector.tensor_mul(t1, qc, cos_sb[:, s0 : s0 + ST])
                    t2 = smalls.tile([P, ST], f16, tag="t2")
                    nc.vector.tensor_mul(t2, rc, sin_sb[:, s0 : s0 + ST])
                    nc.vector.tensor_add(dst[:, dsti, s0 : s0 + ST], t1, t2)

                for si in range(NSB):
                    s0 = si * ST
                    if si in hs_blks:
                        hs_blk = hs_blks[si]
                    else:
                        hs_blk = hsp.tile([P, KT, ST], f16, tag="hs")
                        nc.scalar.dma_start(out=hs_blk, in_=hsT[:, si, :, :])

                    def proj(w_slice, dst, dsti):
                        pm = psA.tile([P, ST], f32, tag="ps")
                        for kt in range(KT):
                            nc.tensor.matmul(
                                pm,
                                lhsT=w_slice[:, kt, :],
                                rhs=hs_blk[:, kt, :],
                                start=(kt == 0),
                                stop=(kt == KT - 1),
                            )
                        qc = smalls.tile([P, ST], f16, tag="qc")
                        nc.vector.tensor_copy(qc, pm)
                        pending.append((qc, s0, dst, dsti))

                    # v first: needs only hs + the small wv
                    for sj in range(ST // P):
                        tt = s0 // P + sj
                        pv = psA.tile([P, NKV_L * HD], f32, tag="ps")
                        for kt in range(KT):
                            nc.tensor.matmul(
                                pv,
                                lhsT=hs_blk[:, kt, sj * P : (sj + 1) * P],
                                rhs=wv_sb[:, kt, :],
                                start=(kt == 0),
                                stop=(kt == KT - 1),
                            )
                        nc.scalar.copy(v_sb[:, tt, :], pv)
                    for j in range(NKV_L):
                        proj(wk_sb[:, j], k_sb, j)
                        if len(pending) > 1:
                            rope_flush()
                    for h in range(NH_L):
                        proj(wq_sb[:, h], q_sb, h)
                        if len(pending) > 1:
                            rope_flush()
                while pending:
                    rope_flush()

            # ---------------- Phase B: attention + interleaved o_proj ------
            with (
                tc.tile_pool(name="wo", bufs=1) as wop,
                tc.tile_pool(name="attn", bufs=1) as ap_,
                tc.tile_pool(name="pblk", bufs=2) as pp,
                tc.tile_pool(name="phalf", bufs=2) as php,
                tc.tile_pool(name="rcps", bufs=4) as rcpp,
                tc.tile_pool(name="outp", bufs=2) as op_,
                tc.tile_pool(name="psB", bufs=4, space="PSUM") as psB,
                tc.tile_pool(name="psc", bufs=2, space="PSUM") as pscp,
            ):
                wo_sb = wop.tile([P, NH_L, HID], f16, tag="wo")
                nc.sync.dma_start(out=wo_sb, in_=wo)
                attnT = ap_.tile([P, NH_L, S], f16, tag="attnT")
                QT = TT // 4

                def scores(h, si, pblk):
                    j = h // GROUP
                    s0 = si * ST
                    for t2_ in range(TT // 2):
                        psc = pscp.tile([P, 2, ST], f32, tag="psc")
                        for u in range(2):
                            tt = 2 * t2_ + u
                            nc.tensor.matmul(
                                psc[:, u, :],
                                lhsT=k_sb[:, j, tt * P : (tt + 1) * P],
                                rhs=q_sb[:, h, s0 : s0 + ST],
                                start=True,
                                stop=True,
                            )
                        nc.scalar.activation(
                            out=pblk[:, 2 * t2_ : 2 * t2_ + 2, :],
                            in_=psc,
                            func=EXP,
                            scale=SCALE,
                        )

                def post(h, si, pblk):
                    j = h // GROUP
                    s0 = si * ST
                    # two fp16 tree levels -> 4 tiles for the ones-matmul
                    ph = php.tile([P, TT // 2, ST], f16, tag="ph")
                    nc.vector.tensor_add(
                        ph, pblk[:, : TT // 2, :], pblk[:, TT // 2 :, :]
                    )
                    nc.vector.tensor_add(
                        ph[:, :QT, :], ph[:, :QT, :], ph[:, QT : 2 * QT, :]
                    )
                    nc.vector.tensor_add(
                        ph[:, : QT // 2, :],
                        ph[:, : QT // 2, :],
                        ph[:, QT // 2 : QT, :],
                    )
                    pat = psB.tile([P, ST], f32, tag="ps")
                    for tt in range(TT):
                        nc.tensor.matmul(
                            pat,
                            lhsT=v_sb[:, tt, j * HD : (j + 1) * HD],
                            rhs=pblk[:, tt, :],
                            start=(tt == 0),
                            stop=(tt == TT - 1),
                        )
                    pcs = psB.tile([P, ST], f32, tag="ps")
                    for tt in range(QT // 2):
                        nc.tensor.matmul(
                            pcs,
                            lhsT=ones,
                            rhs=ph[:, tt, :],
                            start=(tt == 0),
                            stop=(tt == QT // 2 - 1),
                        )
                    rcp = rcpp.tile([P, ST], f32, tag="rcp")
                    nc.vector.reciprocal_approx_fast(out=rcp, in_=pcs)
                    nc.vector.tensor_mul(attnT[:, h, s0 : s0 + ST], pat, rcp)

                def o_proj(si):
                    for sj in range(ST // P):
                        st = si * (ST // P) + sj
                        ob = op_.tile([P, HID], f32, tag="ob")
                        for ni in range(HID // ST):
                            po = psB.tile([P, ST], f32, tag="ps")
                            for ft in range(NH_L):
                                nc.tensor.matmul(
                                    po,
                                    lhsT=attnT[:, ft, st * P : (st + 1) * P],
                                    rhs=wo_sb[:, ft, ni * ST : (ni + 1) * ST],
                                    start=(ft == 0),
                                    stop=(ft == NH_L - 1),
                                )
                            nc.scalar.copy(ob[:, ni * ST : (ni + 1) * ST], po)
                        nc.sync.dma_start(out=out[st * P : (st + 1) * P, :], in_=ob)

                prev = None
                for si in range(NSB):
                    for h in range(NH_L):
                        pblk = pp.tile([P, TT, ST], f16, tag="pblk")
                        scores(h, si, pblk)
                        if prev is not None:
                            post(*prev)
                        prev = (h, si, pblk)
                    post(*prev)
                    prev = None
                    o_proj(si)

    nc.compile()
    return nc


def _get_nc():
    if "nc" not in _CACHE:
        _CACHE["nc"] = _build()
    return _CACHE["nc"]


def kernel(hidden_states, cos, sin, Wq, Wk, Wv, Wo):
    if "/opt/trn_rl_repo" not in sys.path:
        sys.path.insert(0, "/opt/trn_rl_repo")
    from concourse.bass_utils import run_bass_kernel_spmd

    hidden_states = np.asarray(hidden_states, dtype=np.float32)
    cos = np.asarray(cos, dtype=np.float32)
    sin = np.asarray(sin, dtype=np.float32)
    Wq = np.asarray(Wq, dtype=np.float32)
    Wk = np.asarray(Wk, dtype=np.float32)
    Wv = np.asarray(Wv, dtype=np.float32)
    Wo = np.asarray(Wo, dtype=np.float32)

    nc = _get_nc()
    rm = _rot_lhsT()

    # pretiled host layouts: partition index first, contiguous per DMA slice
    def tile_khid(w):  # [HID, F] -> [P, KT, F]
        return np.ascontiguousarray(
            w.reshape(KT, P, w.shape[1]).transpose(1, 0, 2)
        ).astype(np.float16)

    in_maps = []
    hsT_b = [
        np.ascontiguousarray(
            hidden_states[b].T.reshape(KT, P, NSB, ST).transpose(1, 2, 0, 3)
        ).astype(np.float16)
        for b in range(B)
    ]
    cosT_b = [np.ascontiguousarray(cos[b].T).astype(np.float16) for b in range(B)]
    sinT_b = [np.ascontiguousarray(sin[b].T).astype(np.float16) for b in range(B)]
    for c in range(2 * B):
        b, half = c // 2, c % 2
        fq = slice(half * NH_L * HD, (half + 1) * NH_L * HD)
        fkv = slice(half * NKV_L * HD, (half + 1) * NKV_L * HD)
        wq_t = tile_khid(Wq[:, fq]).reshape(P, KT, NH_L, HD).transpose(0, 2, 1, 3)
        wk_t = tile_khid(Wk[:, fkv]).reshape(P, KT, NKV_L, HD).transpose(0, 2, 1, 3)
        wo_t = np.ascontiguousarray(
            Wo[fq, :].reshape(NH_L, P, HID).transpose(1, 0, 2)
        ).astype(np.float16)
        in_maps.append(
            {
                "hsT": hsT_b[b],
                "wq": np.ascontiguousarray(wq_t),
                "wk": np.ascontiguousarray(wk_t),
                "wv": tile_khid(Wv[:, fkv]),
                "wo": wo_t,
                "cosT": cosT_b[b],
                "sinT": sinT_b[b],
                "rmat": rm,
            }
        )

    res = run_bass_kernel_spmd(nc, in_maps, list(range(2 * B)))
    _CACHE["last_results"] = res

    out = np.empty((B, S, HID), dtype=np.float32)
    for b in range(B):
        out[b] = res.results[2 * b]["out"] + res.results[2 * b + 1]["out"]
    return out


# revision 10
# speedup vs baseline: 1.2981x; 1.0035x over previous
"""GQA attention (RoPE + softmax + o_proj) on 8 Trainium2 NeuronCores.

Problem shapes (hardcoded): hidden_states [4, 2048, 2048], 16 q heads,
4 kv heads, head_dim 128, rope cos/sin tables given as inputs.

Sharding: core c -> (batch b = c // 2, q-head half = c % 2).  Each core
computes 8 q heads + their 2 kv heads for one batch and produces a
partial o_proj output [2048, 2048]; the host sums the two halves per
batch (tensor parallel, no device collectives).

All matmuls run in fp16 (1 cycle/row on PE) with fp32 PSUM accumulation:
  - q^T/k^T = W^T @ hs^T with hid on partitions (weights are natural lhsT)
  - RoPE via a +-1 permutation matmul (rotate_half), software-pipelined
    one projection behind so PE never waits on the PSUM copyback
  - scores^T[t, s] with k^T tiles stationary; exp via ScalarE (fused
    1/sqrt(d) scale) reads two PSUM banks per instruction and writes
    P^T fp16 straight to SBUF; the exp-dependent stages of iteration i
    are emitted after the score matmuls of iteration i+1 (software
    pipeline) so PE never idles waiting for ScalarE
  - softmax denominators: two fp16 DVE pairwise adds over P^T quarters,
    then an all-ones stationary matmul (result replicated across
    partitions = pre-broadcast), fast DVE reciprocal, fused
    normalize+cast on the attn PSUM->SBUF copyback
  - attn^T[d, s] = v-tiles stationary @ P^T; o_proj with attn^T tiles
    stationary over Wo, interleaved per s-block right after the heads
    of that block finish.
"""

import sys

import numpy as np

B, S, HID = 4, 2048, 2048
NH, NKV, HD = 16, 4, 128
NH_L = 8        # q heads per core
NKV_L = 2       # kv heads per core
GROUP = NH // NKV
P = 128
ST = 512        # s-block (matmul free dim)
NSB = S // ST   # 4 s-blocks
KT = HID // P   # 16 contraction tiles over hidden
TT = S // P     # 16 key/t tiles
SCALE = 1.0 / float(np.sqrt(HD))

_CACHE = {}


def _rot_lhsT():
    """Stationary operand R such that R.T @ q^T = rotate_half(q)^T."""
    r = np.zeros((HD, HD), dtype=np.float16)
    half = HD // 2
    i = np.arange(half)
    r[i + half, i] = -1.0
    r[i, i + half] = 1.0
    return r


def _build():
    if "/opt/trn_rl_repo" not in sys.path:
        sys.path.insert(0, "/opt/trn_rl_repo")
    import concourse.mybir as mybir
    from concourse import bacc
    from concourse.tile import TileContext
    from concourse.tile_rust import add_dep_helper

    dt = mybir.dt
    f16, f32 = dt.float16, dt.float32

    nc = bacc.Bacc("TRN2", target_bir_lowering=False, debug=False, num_devices=8)
    # host-pretiled layouts (see kernel() below)
    hsT = nc.dram_tensor("hsT", [P, NSB, KT, ST], f16, kind="ExternalInput").ap()
    wq = nc.dram_tensor("wq", [P, NH_L, KT, HD], f16, kind="ExternalInput").ap()
    wk = nc.dram_tensor("wk", [P, NKV_L, KT, HD], f16, kind="ExternalInput").ap()
    wv = nc.dram_tensor("wv", [P, KT, NKV_L * HD], f16, kind="ExternalInput").ap()
    wo = nc.dram_tensor("wo", [P, NH_L, HID], f16, kind="ExternalInput").ap()
    cosT = nc.dram_tensor("cosT", [HD, S], f16, kind="ExternalInput").ap()
    sinT = nc.dram_tensor("sinT", [HD, S], f16, kind="ExternalInput").ap()
    rmat = nc.dram_tensor("rmat", [HD, HD], f16, kind="ExternalInput").ap()
    out = nc.dram_tensor("out", [S, HID], f32, kind="ExternalOutput").ap()

    EXP = mybir.ActivationFunctionType.Exp

    with TileContext(nc) as tc:
        with (
            tc.tile_pool(name="consts", bufs=1) as consts,
            tc.tile_pool(name="qkv", bufs=1) as qkvp,
        ):
            ones = consts.tile([P, P], f16, tag="ones")
            nc.vector.memset(ones, 1.0)
            rot = consts.tile([HD, HD], f16, tag="rot")
            nc.gpsimd.dma_start(out=rot, in_=rmat)

            q_sb = qkvp.tile([P, NH_L, S], f16, tag="q")
            k_sb = qkvp.tile([P, NKV_L, S], f16, tag="k")
            v_sb = qkvp.tile([P, TT, NKV_L * HD], f16, tag="v")

            # ---------------- Phase A: projections + RoPE ----------------
            with (
                tc.tile_pool(name="wqkv", bufs=1) as wp,
                tc.tile_pool(name="trig", bufs=1) as trig,
                tc.tile_pool(name="hs", bufs=2) as hsp,
                tc.tile_pool(name="ropes", bufs=4) as smalls,
                tc.tile_pool(name="psA", bufs=6, space="PSUM") as psA,
            ):
                # hs block 0 first (its consumers are the head of the program)
                hs_blks = {}
                hs_first = hsp.tile([P, KT, ST], f16, tag="hs")
                hs0_dma = nc.scalar.dma_start(out=hs_first, in_=hsT[:, 0, :, :])
                hs_blks[0] = hs_first

                wv_sb = wp.tile([P, KT, NKV_L * HD], f16, tag="wv")
                nc.gpsimd.dma_start(out=wv_sb, in_=wv)
                cos_sb = trig.tile([HD, S], f16, tag="cos")
                nc.gpsimd.dma_start(out=cos_sb, in_=cosT)
                sin_sb = trig.tile([HD, S], f16, tag="sin")
                nc.gpsimd.dma_start(out=sin_sb, in_=sinT)

                wk_sb = wp.tile([P, NKV_L, KT, HD], f16, tag="wk")
                nc.sync.dma_start(out=wk_sb, in_=wk)
                wq_sb = wp.tile([P, NH_L, KT, HD], f16, tag="wq")
                for h in range(NH_L):  # per-head DMAs so head 0 starts early
                    wqd = nc.sync.dma_start(out=wq_sb[:, h, :, :], in_=wq[:, h, :, :])
                    # keep HBM bandwidth free for the first-needed tensors
                    add_dep_helper(
                        wqd.ins, hs0_dma.ins, sync=True, reason="defer wq behind hs0"
                    )

                # software pipeline: the rot-matmul + rope combine for one
                # projection is emitted while the NEXT projection's matmul
                # group runs, so PE never waits on the PSUM copyback.
                pending = []

                def rope_flush():
                    qc, s0, dst, dsti = pending.pop(0)
                    pr = psA.tile([P, ST], f32, tag="ps")
                    nc.tensor.matmul(pr, lhsT=rot, rhs=qc, start=True, stop=True)
                    rc = smalls.tile([P, ST], f16, tag="rc")
                    nc.vector.tensor_copy(rc, pr)
                    t1 = smalls.tile([P, ST], f16, tag="t1")
                    nc.vector.tensor_mul(t1, qc, cos_sb[:, s0 : s0 + ST])
                    t2 = smalls.tile([P, ST], f16, tag="t2")
                    nc.vector.tensor_mul(t2, rc, sin_sb[:, s0 : s0 + ST])
                    nc.vector.tensor_add(dst[:, dsti, s0 : s0 + ST], t1, t2)

                for si in range(NSB):
                    s0 = si * ST
                    if si in hs_blks:
                        hs_blk = hs_blks[si]
                    else:
                        hs_blk = hsp.tile([P, KT, ST], f16, tag="hs")
                        nc.scalar.dma_start(out=hs_blk, in_=hsT[:, si, :, :])

                    def proj(w_slice, dst, dsti):
                        pm = psA.tile([P, ST], f32, tag="ps")
                        for kt in range(KT):
                            nc.tensor.matmul(
                                pm,
                                lhsT=w_slice[:, kt, :],
                                rhs=hs_blk[:, kt, :],
                                start=(kt == 0),
                                stop=(kt == KT - 1),
                            )
                        qc = smalls.tile([P, ST], f16, tag="qc")
                        nc.vector.tensor_copy(qc, pm)
                        pending.append((qc, s0, dst, dsti))

                    # v first: needs only hs + the small wv
                    for sj in range(ST // P):
                        tt = s0 // P + sj
                        pv = psA.tile([P, NKV_L * HD], f32, tag="ps")
                        for kt in range(KT):
                            nc.tensor.matmul(
                                pv,
                                lhsT=hs_blk[:, kt, sj * P : (sj + 1) * P],
                                rhs=wv_sb[:, kt, :],
                                start=(kt == 0),
                                stop=(kt == KT - 1),
                            )
                        nc.scalar.copy(v_sb[:, tt, :], pv)
                    for j in range(NKV_L):
                        proj(wk_sb[:, j], k_sb, j)
                        if len(pending) > 1:
                            rope_flush()
                    for h in range(NH_L):
                        proj(wq_sb[:, h], q_sb, h)
                        if len(pending) > 1:
                            rope_flush()
                while pending:
                    rope_flush()

            # ---------------- Phase B: attention + interleaved o_proj ------
            with (
                tc.tile_pool(name="wo", bufs=1) as wop,
                tc.tile_pool(name="attn", bufs=1) as ap_,
                tc.tile_pool(name="pblk", bufs=2) as pp,
                tc.tile_pool(name="phalf", bufs=2) as php,
                tc.tile_pool(name="rcps", bufs=4) as rcpp,
                tc.tile_pool(name="outp", bufs=2) as op_,
                tc.tile_pool(name="psB", bufs=4, space="PSUM") as psB,
                tc.tile_pool(name="psc", bufs=2, space="PSUM") as pscp,
            ):
                wo_sb = wop.tile([P, NH_L, HID], f16, tag="wo")
                nc.sync.dma_start(out=wo_sb, in_=wo)
                attnT = ap_.tile([P, NH_L, S], f16, tag="attnT")
                QT = TT // 4

                def scores(h, si, pblk):
                    j = h // GROUP
                    s0 = si * ST
                    for t2_ in range(TT // 2):
                        psc = pscp.tile([P, 2, ST], f32, tag="psc")
                        for u in range(2):
                            tt = 2 * t2_ + u
                            nc.tensor.matmul(
                                psc[:, u, :],
                                lhsT=k_sb[:, j, tt * P : (tt + 1) * P],
                                rhs=q_sb[:, h, s0 : s0 + ST],
                                start=True,
                                stop=True,
                            )
                        nc.scalar.activation(
                            out=pblk[:, 2 * t2_ : 2 * t2_ + 2, :],
                            in_=psc,
                            func=EXP,
                            scale=SCALE,
                        )

                def post(h, si, pblk):
                    j = h // GROUP
                    s0 = si * ST
                    # two fp16 tree levels -> 4 tiles for the ones-matmul
                    ph = php.tile([P, TT // 2, ST], f16, tag="ph")
                    nc.vector.tensor_add(
                        ph, pblk[:, : TT // 2, :], pblk[:, TT // 2 :, :]
                    )
                    nc.vector.tensor_add(
                        ph[:, :QT, :], ph[:, :QT, :], ph[:, QT : 2 * QT, :]
                    )
                    nc.vector.tensor_add(
                        ph[:, : QT // 2, :],
                        ph[:, : QT // 2, :],
                        ph[:, QT // 2 : QT, :],
                    )
                    pat = psB.tile([P, ST], f32, tag="ps")
                    for tt in range(TT):
                        nc.tensor.matmul(
                            pat,
                            lhsT=v_sb[:, tt, j * HD : (j + 1) * HD],
                            rhs=pblk[:, tt, :],
                            start=(tt == 0),
                            stop=(tt == TT - 1),
                        )
                    pcs = psB.tile([P, ST], f32, tag="ps")
                    for tt in range(QT // 2):
                        nc.tensor.matmul(
                            pcs,
                            lhsT=ones,
                            rhs=ph[:, tt, :],
                            start=(tt == 0),
                            stop=(tt == QT // 2 - 1),
                        )
                    rcp = rcpp.tile([P, ST], f32, tag="rcp")
                    nc.vector.reciprocal_approx_fast(out=rcp, in_=pcs)
                    nc.vector.tensor_mul(attnT[:, h, s0 : s0 + ST], pat, rcp)

                def o_proj(si):
                    for sj in range(ST // P):
                        st = si * (ST // P) + sj
                        ob = op_.tile([P, HID], f32, tag="ob")
                        for ni in range(HID // ST):
                            po = psB.tile([P, ST], f32, tag="ps")
                            for ft in range(NH_L):
                                nc.tensor.matmul(
                                    po,
                                    lhsT=attnT[:, ft, st * P : (st + 1) * P],
                                    rhs=wo_sb[:, ft, ni * ST : (ni + 1) * ST],
                                    start=(ft == 0),
                                    stop=(ft == NH_L - 1),
                                )
                            nc.scalar.copy(ob[:, ni * ST : (ni + 1) * ST], po)
                        nc.sync.dma_start(out=out[st * P : (st + 1) * P, :], in_=ob)

                prev = None
                for si in range(NSB):
                    for h in range(NH_L):
                        pblk = pp.tile([P, TT, ST], f16, tag="pblk")
                        scores(h, si, pblk)
                        if prev is not None:
                            post(*prev)
                        prev = (h, si, pblk)
                    post(*prev)
                    prev = None
                    o_proj(si)

    nc.compile()
    return nc


def _get_nc():
    if "nc" not in _CACHE:
        _CACHE["nc"] = _build()
    return _CACHE["nc"]


def kernel(hidden_states, cos, sin, Wq, Wk, Wv, Wo):
    if "/opt/trn_rl_repo" not in sys.path:
        sys.path.insert(0, "/opt/trn_rl_repo")
    from concourse.bass_utils import run_bass_kernel_spmd

    hidden_states = np.asarray(hidden_states, dtype=np.float32)
    cos = np.asarray(cos, dtype=np.float32)
    sin = np.asarray(sin, dtype=np.float32)
    Wq = np.asarray(Wq, dtype=np.float32)
    Wk = np.asarray(Wk, dtype=np.float32)
    Wv = np.asarray(Wv, dtype=np.float32)
    Wo = np.asarray(Wo, dtype=np.float32)

    nc = _get_nc()
    rm = _rot_lhsT()

    # pretiled host layouts: partition index first, contiguous per DMA slice
    def tile_khid(w):  # [HID, F] -> [P, KT, F]
        return np.ascontiguousarray(
            w.reshape(KT, P, w.shape[1]).transpose(1, 0, 2)
        ).astype(np.float16)

    in_maps = []
    hsT_b = [
        np.ascontiguousarray(
            hidden_states[b].T.reshape(KT, P, NSB, ST).transpose(1, 2, 0, 3)
        ).astype(np.float16)
        for b in range(B)
    ]
    cosT_b = [np.ascontiguousarray(cos[b].T).astype(np.float16) for b in range(B)]
    sinT_b = [np.ascontiguousarray(sin[b].T).astype(np.float16) for b in range(B)]
    for c in range(2 * B):
        b, half = c // 2, c % 2
        fq = slice(half * NH_L * HD, (half + 1) * NH_L * HD)
        fkv = slice(half * NKV_L * HD, (half + 1) * NKV_L * HD)
        wq_t = tile_khid(Wq[:, fq]).reshape(P, KT, NH_L, HD).transpose(0, 2, 1, 3)
        wk_t = tile_khid(Wk[:, fkv]).reshape(P, KT, NKV_L, HD).transpose(0, 2, 1, 3)
        wo_t = np.ascontiguousarray(
            Wo[fq, :].reshape(NH_L, P, HID).transpose(1, 0, 2)
        ).astype(np.float16)
        in_maps.append(
            {
                "hsT": hsT_b[b],
                "wq": np.ascontiguousarray(wq_t),
                "wk": np.ascontiguousarray(wk_t),
                "wv": tile_khid(Wv[:, fkv]),
                "wo": wo_t,
                "cosT": cosT_b[b],
                "sinT": sinT_b[b],
                "rmat": rm,
            }
        )

    res = run_bass_kernel_spmd(nc, in_maps, list(range(2 * B)))
    _CACHE["last_results"] = res

    out = np.empty((B, S, HID), dtype=np.float32)
    for b in range(B):
        out[b] = res.results[2 * b]["out"] + res.results[2 * b + 1]["out"]
    return out
